# revision 35
# baseline (speedup 1.0000x reference)
"""MoE-routed group-norm kernel for Trainium2 (Bass/Tile), 8-core SPMD.

Problem (hardcoded shapes):
  x: [64, 512, 32, 32] f32
  experts_weight/bias: [8, 512], shared_weight/bias: [512]
  router_w: [8, 512], router_b: [8]

  flat = x.mean((2,3)); logits = flat @ router_w.T + router_b
  prob = softmax(logits); top-2 -> coeff = vals / sum(vals)
  fused_w = sum_k coeff_k * experts_weight[idx_k] + shared_weight (bias likewise)
  group-norm over G=32 groups of 16 channels, then y = x_norm * fused_w + fused_b

Strategy: data-parallel over batch, 8 samples per core, channels on
partitions ([512,1024] = 4 chunks of [128,1024] per sample, channel
c = 128*j + p so each chunk holds 8 complete norm groups of 16 partitions).

Two full-tensor passes only (the memory roofline allows ~82us; everything
else must hide under it):
  S (stats): DVE bn_stats, 2x [128,512] per chunk -> (count, mean, n*var)
     for even/odd element sets. One pass yields BOTH sum(x) and sum(x^2):
     s1 = sum of 4 means (= sum x / 256), s2 = sum(n*var) + 256*sum(mean^2).
     This removes the baseline's separate ACT Square pass entirely.
  P (apply): y = A*x + B split ACT (chunks 0,1) / GpSimd (chunks 2,3),
     keeping DVE free for the stats of the next pair.

All cross-partition math is batched per PAIR of samples into 7 PE matmuls:
4 chained logits matmuls (K-tiled over chunks), 1 group-sum matmul (gmask
against the packed [s1|s2] tile), 1 coeff transpose [2,8]->[8,2], and ONE
merged [72,128]-stationary matmul that simultaneously broadcasts group
mean/rstd to channels (bmask rows) and mixes the expert weight/bias tables
(block-diagonal coeff moving operand) -- replacing the baseline's 9
broadcast+mix matmuls per pair.

Routing runs in a [2, E] layout: top-1 exp is exactly 1.0 so is_lt/is_ge
masking replaces index math; rstd = bit-trick seed + 2 Newton steps on DVE
(keeps ACT's table pinned to exp_and_others; Sqrt lives in another set).

DMA: loads on the SP HWDGE ring (2 x 1 MiB per sample), stores split per
producing engine -- ACT's chunks leave on the ACT HWDGE ring, GpSimd's on
the SWDGE ring -- so no engine ever stalls waiting on another engine's
pass2 to dispatch a store. The loop is software-pipelined with a 1-pair
lag (stats of pair i+1 issue before routing/apply of pair i) so DVE never
idles waiting for PE logits.
"""

import numpy as np

import concourse.bacc as bacc
import concourse.bass as bass
import concourse.tile as tile
from concourse import mybir
from concourse.bass_utils import run_bass_kernel_spmd

F32 = mybir.dt.float32
BF16 = mybir.dt.bfloat16
I32 = mybir.dt.int32
ALU = mybir.AluOpType
ACTF = mybir.ActivationFunctionType
AXX = mybir.AxisListType.X
AXY = mybir.AxisListType.XY

P = 128            # SBUF partitions
B, C, HWD = 64, 512, 1024
E, G = 8, 32
EPS = 1e-5
NCORES = 8
BPC = B // NCORES  # samples per core
NCH = C // P       # 4 channel chunks per sample
CPG = C // G       # 16 channels per group
PAIR = 2
RSQRT_MAGIC = 0x5F3759DF

# ca layout [128, 56]:
#   0:32  routerT/4  (ca[p, 8j+e] = router_w[e, 128j+p] / 4; logits use
#                     s1 = sum(x)/256 so the net scale is 1/1024)
#   32:40 gmask      (1 if p//16 == g)
#   40:48 sw2        ((j, s): shared_weight[128j+p], replicated over s)
#   48:56 sb2        (same for shared_bias)
CA_W = 56
# cb layout [64, 282] (everything at partition base 0 -- the BIR verifier
# rejects engine APs starting at unaligned partitions):
#   [:, 0:128]     T: row 8*(4u+j)+e = table_u[e, 128j+p]  (u: 0=w, 1=b)
#   [0:8, 128:256] bmask[g, p] (1 if p//16 == g)
#   [:, 256:272]   dmask: row r=(k,e), col 2*c2+s = 1 iff c2 == k
#   [0:1, 272:280] router bias (row 0)
#   [0:2, 280:282] 2x2 identity (coeff transpose moving operand)
#   [0:1, 282:284] ones (bias-add matmul stationary)
CB_W = 284


def build(n_b: int = BPC) -> bass.Bass:
    assert n_b % PAIR == 0
    npair = n_b // PAIR
    nc = bacc.Bacc()
    x_d = nc.declare_dram_parameter("x", [n_b, C, HWD], F32, isOutput=False)
    ca_d = nc.declare_dram_parameter("ca", [P, CA_W], F32, isOutput=False)
    cb_d = nc.declare_dram_parameter("cb", [64, CB_W], F32, isOutput=False)
    y_d = nc.declare_dram_parameter("y", [n_b, C, HWD], BF16, isOutput=True)

    with tile.TileContext(nc) as tc:
        with (
            tc.tile_pool(name="consts", bufs=1) as consts,
            tc.tile_pool(name="xp", bufs=6) as xp,
            tc.tile_pool(name="yp", bufs=6) as yp,
            tc.tile_pool(name="stp", bufs=3) as stp,
            tc.tile_pool(name="scrp", bufs=1) as scrp,
            tc.tile_pool(name="ssp", bufs=4) as ssp,
            tc.tile_pool(name="mvp", bufs=2) as mvp,
            tc.tile_pool(name="tinyp", bufs=2) as tinyp,
            tc.tile_pool(name="ps_static", bufs=1, space="PSUM") as pstat,
        ):
            # consts staged through a DVE copy so PE inputs have DVE
            # provenance; the copies are issued later (in stats_tail(0)) so
            # the first sample's bn_stats never queue behind them on DVE
            ca_st = consts.tile([P, CA_W], F32)
            nc.scalar.dma_start(out=ca_st, in_=ca_d[:, :])
            cb_st = consts.tile([64, CB_W], F32)
            nc.scalar.dma_start(out=cb_st, in_=cb_d[:, :])
            ca = consts.tile([P, CA_W], F32)
            cb = consts.tile([64, CB_W], F32)
            magic8 = consts.tile([E, PAIR * NCH], F32)
            one8i = consts.tile([E, PAIR * NCH], F32)

            gmask = ca[:, 32:40]
            sw2 = ca[:, 40:48].rearrange("p (j s) -> p j s", s=PAIR)
            sb2 = ca[:, 48:56].rearrange("p (j s) -> p j s", s=PAIR)
            tab64 = cb[:, 0:P]
            bmask = cb[0:E, P : P + P]
            dmask = cb[:, 256:272]
            rb_row = cb[0:1, 272:280]
            ident2 = cb[0:PAIR, 280:282]
            one2 = cb[0:1, 282:284]

            # static per-pair PSUM regions (never reused -> no PSUM WAW deps)
            ps_lg = pstat.tile([PAIR, E * npair], F32, tag="lg")
            ps_gs = pstat.tile([E, 16 * npair], F32, tag="gs")
            ps_ct = pstat.tile([64, PAIR * npair], F32, tag="ct")
            ps_bc = pstat.tile([P, 32 * npair], F32, tag="bc")

            xts: dict[int, object] = {}
            sts: dict[int, object] = {}
            sss: dict[int, object] = {}
            erows: dict[int, object] = {}

            def stats_sample(b: int):
                ip, bb = divmod(b, PAIR)
                if bb == 0:
                    st_t = stp.tile([P, 3, PAIR, NCH, 2, 2], F32, tag="st")
                    ss_t = ssp.tile([P, PAIR, NCH, 2], F32, tag="ss")
                    sts[ip] = st_t
                    sss[ip] = ss_t
                st = sts[ip]
                x_t = xp.tile([P, NCH, HWD], F32, tag="x")
                xts[b] = x_t
                xv = x_d[b].rearrange("(t p) f -> p t f", p=P)
                # 512 KB per-chunk loads: each chunk's bn_stats starts the
                # moment its quarter lands
                for j in range(NCH):
                    nc.sync.dma_start(
                        out=x_t[:, j : j + 1, :], in_=xv[:, j : j + 1, :]
                    )
                for j in range(3):
                    for h in range(2):
                        # out AP scatters (eo, field) into field-major planes
                        # so the conv reads below are contiguous
                        nc.vector.bn_stats(
                            st[:, :, bb, j, h, :].rearrange("p f eo -> p eo f"),
                            x_t[:, j, h * 512 : (h + 1) * 512],
                        )
                # chunk 3's stats ride ACT (identity/square + accum_out),
                # writing s1/s2 straight into ss -- offloads 2 of 8 bn_stats
                # from the pacing engine (DVE)
                ss = sss[ip]
                jk1 = scrp.tile([P, HWD], F32, tag="jk")
                nc.scalar.activation(
                    jk1,
                    x_t[:, 3, :],
                    ACTF.Identity,
                    scale=1.0 / 256,
                    accum_out=ss[:, bb, 3, 0:1],
                )
                jk2 = scrp.tile([P, HWD], F32, tag="jk")
                nc.scalar.activation(
                    jk2, x_t[:, 3, :], ACTF.Square, accum_out=ss[:, bb, 3, 1:2]
                )

            def stats_tail(ip: int):
                if ip == 0:
                    nc.vector.tensor_copy(ca, ca_st)
                    nc.vector.tensor_copy(cb, cb_st)
                    nc.vector.memset(magic8[:, :].bitcast(I32), RSQRT_MAGIC)
                    nc.vector.memset(one8i[:, :].bitcast(I32), 1)
                st = sts.pop(ip)
                ss = sss[ip]
                # conv (chunks 0..2; chunk 3 came via ACT accum):
                # s1 = sum of the 4 means; s2 = sum(cv) + 256*sum(mean^2)
                mv = st[:, 1, :, 0:3, :, :]   # [P, b, j, h, eo] contiguous
                cv = st[:, 2, :, 0:3, :, :]
                nc.vector.reduce_sum(ss[:, :, 0:3, 0], mv, axis=AXY)
                sq = tinyp.tile([P, PAIR, 3, 2, 2], F32, tag="sq")
                nc.vector.tensor_tensor(sq, mv, mv, ALU.mult)
                msq = tinyp.tile([P, PAIR, 3], F32, tag="msq")
                nc.vector.reduce_sum(msq, sq, axis=AXY)
                scv = tinyp.tile([P, PAIR, 3], F32, tag="scv")
                nc.vector.reduce_sum(scv, cv, axis=AXY)
                nc.vector.scalar_tensor_tensor(
                    ss[:, :, 0:3, 1], msq, 256.0, scv, op0=ALU.mult, op1=ALU.add
                )
                # PE: logits (K-tiled over chunks, + bias row) and group
                # sums of [s1|s2]
                lg = ps_lg[:, E * ip : E * (ip + 1)]
                for j in range(NCH):
                    nc.tensor.matmul(
                        lg,
                        ss[:, :, j, 0],
                        ca[:, j * 8 : (j + 1) * 8],
                        start=(j == 0),
                        stop=False,
                    )
                nc.tensor.matmul(lg, one2, rb_row, start=False, stop=True)
                nc.tensor.matmul(
                    ps_gs[:, 16 * ip : 16 * (ip + 1)], gmask, ss[:, :, :, :]
                )
                # exp issued here (reads logits PSUM directly; |logits| is
                # O(0.3) so no max-subtraction needed) so it lands on ACT's
                # queue BEFORE the next pair's pass2 -- the DVE routing ops
                # in apply_phase never wait on ACT's pass2 backlog
                erow_t = tinyp.tile([PAIR, E], F32, tag="erow")
                nc.scalar.activation(erow_t, lg, ACTF.Exp)
                erows[ip] = erow_t

            def apply_phase(ip: int):
                erow = erows.pop(ip)
                # routing, pair-batched in [2, E] partition layout: the DVE
                # top-8 sort gives (v1, v2) in one op
                mx = tinyp.tile([PAIR, 8], F32, tag="mx")
                nc.vector.max(mx, erow)
                gate = tinyp.tile([PAIR, E], F32, tag="gate")
                nc.vector.scalar_tensor_tensor(
                    gate, erow, mx[:, 1:2], erow, op0=ALU.is_ge, op1=ALU.mult
                )
                den = tinyp.tile([PAIR, 1], F32, tag="den")
                nc.vector.tensor_tensor(den, mx[:, 0:1], mx[:, 1:2], ALU.add)
                rden = tinyp.tile([PAIR, 1], F32, tag="rden")
                nc.vector.reciprocal(rden, den)
                # crow64[s, 8k+e] = coeff[s, e] replicated over k=0..7 via a
                # stride-0 broadcast read; the PE transpose then yields
                # ctb[(k,e), s] at partition base 0
                crow64 = tinyp.tile([PAIR, 64], F32, tag="crow64")
                c64v = crow64[:, :].rearrange("s (k e) -> s k e", k=8)
                gate_b, _ = bass.broadcast_tensor_aps(
                    gate.rearrange("s (o e) -> s o e", o=1), c64v
                )
                nc.vector.tensor_scalar_mul(c64v, gate_b, rden[:, 0:1])
                ct = ps_ct[:, PAIR * ip : PAIR * (ip + 1)]
                nc.tensor.matmul(ct, crow64, ident2)

                # moving operands for the split broadcast+mix matmuls
                M1 = mvp.tile([64, 16], F32, tag="mv")
                mrt = tinyp.tile([E, 16], F32, tag="mrt")
                mr = mrt[:, 0:16].rearrange("g (b j t) -> g b j t", b=PAIR, j=NCH)
                gs = ps_gs[:, 16 * ip : 16 * (ip + 1)].rearrange(
                    "g (b j t) -> g b j t", b=PAIR, j=NCH
                )
                nc.vector.tensor_scalar_mul(mr[:, :, :, 0], gs[:, :, :, 0], 1.0 / 64)
                ex2 = tinyp.tile([E, PAIR, NCH], F32, tag="ex2")
                nc.vector.tensor_scalar_mul(ex2, gs[:, :, :, 1], 1.0 / 16384)
                mg2 = tinyp.tile([E, PAIR, NCH], F32, tag="mg2")
                nc.vector.tensor_tensor(mg2, mr[:, :, :, 0], mr[:, :, :, 0], ALU.mult)
                v = tinyp.tile([E, PAIR * NCH], F32, tag="v")
                vv = v[:, :].rearrange("g (b j) -> g b j", b=PAIR)
                nc.vector.scalar_tensor_tensor(
                    vv, ex2, EPS, mg2, op0=ALU.add, op1=ALU.subtract
                )
                # rstd = rsqrt(v): bit-trick seed + 1 Newton iteration (DVE);
                # worst-case ~0.18% rel err, far inside the 2e-2 gate
                yr = tinyp.tile([E, PAIR * NCH], F32, tag="yr")
                nc.vector.tensor_tensor(
                    yr[:, :].bitcast(I32),
                    v[:, :].bitcast(I32),
                    one8i[:, :].bitcast(I32),
                    ALU.arith_shift_right,
                )
                nc.vector.tensor_tensor(
                    yr[:, :].bitcast(I32),
                    magic8[:, :].bitcast(I32),
                    yr[:, :].bitcast(I32),
                    ALU.subtract,
                )
                t_a = tinyp.tile([E, PAIR * NCH], F32, tag="t_a")
                t_b = tinyp.tile([E, PAIR * NCH], F32, tag="t_b")
                for _ in range(1):
                    nc.vector.tensor_tensor(t_a, yr, yr, ALU.mult)
                    nc.vector.tensor_tensor(t_b, t_a, v, ALU.mult)
                    nc.vector.tensor_scalar(
                        t_a, t_b, -0.5, 1.5, op0=ALU.mult, op1=ALU.add
                    )
                    nc.vector.tensor_tensor(yr, yr, t_a, ALU.mult)
                nc.vector.tensor_copy(
                    mr[:, :, :, 1], yr[:, :].rearrange("g (b j) -> g b j", b=PAIR)
                )
                # coeff diagonal in one op: M1[(k,e), (c2,s)] = ctb[(k,e), s]
                # (stride-0 broadcast over c2) * dmask (1 iff c2==k)
                m1v = M1[:, :].rearrange("r (c s) -> r c s", c=8)
                ct_b, _ = bass.broadcast_tensor_aps(
                    ct.rearrange("r (o s) -> r o s", o=1), m1v
                )
                nc.vector.tensor_tensor(
                    m1v, ct_b, dmask[:, :].rearrange("r (c s) -> r c s", c=8), ALU.mult
                )

                bc = ps_bc[:, 32 * ip : 32 * (ip + 1)]
                nc.tensor.matmul(bc[:, 16:32], tab64, M1)
                nc.tensor.matmul(bc[:, 0:16], bmask, mrt)

                # A = (fu_w + shared_w) * rstd ; B = (fu_b + shared_b) - mean*A
                bcm = bc[:, 0:16].rearrange("p (b j t) -> p b j t", b=PAIR, j=NCH)
                fut = bc[:, 16:32].rearrange("p (u j s) -> p u j s", u=2, j=NCH)
                bc_mean = bcm[:, :, :, 0].rearrange("p b j -> p j b")
                bc_rstd = bcm[:, :, :, 1].rearrange("p b j -> p j b")
                t1 = tinyp.tile([P, NCH, PAIR], F32, tag="t1")
                nc.vector.tensor_tensor(t1, fut[:, 0, :, :], sw2, ALU.add)
                At = tinyp.tile([P, NCH, PAIR], F32, tag="At")
                nc.vector.tensor_tensor(At, t1, bc_rstd, ALU.mult)
                t2 = tinyp.tile([P, NCH, PAIR], F32, tag="t2")
                nc.vector.tensor_tensor(t2, fut[:, 1, :, :], sb2, ALU.add)
                t3 = tinyp.tile([P, NCH, PAIR], F32, tag="t3")
                nc.vector.tensor_tensor(t3, bc_mean, At, ALU.mult)
                Bt = tinyp.tile([P, NCH, PAIR], F32, tag="Bt")
                nc.vector.tensor_tensor(Bt, t2, t3, ALU.subtract)

                # pass2: j0 on ACT, j1/j2 on GpSimd (adjacent -> one 1 MiB
                # store), j3 on DVE (it paid nothing for chunk 3's stats).
                # For the final pair j1 also moves to DVE to cut the drain.
                last = ip == npair - 1

                def p2(eng, y_t, x_t, j, bb):
                    if eng == "act":
                        nc.scalar.activation(
                            y_t[:, j, :],
                            x_t[:, j, :],
                            ACTF.Identity,
                            bias=Bt[:, j, bb : bb + 1],
                            scale=At[:, j, bb : bb + 1],
                        )
                    else:
                        e = nc.vector if eng == "dve" else nc.gpsimd
                        e.tensor_scalar(
                            y_t[:, j, :],
                            x_t[:, j, :],
                            At[:, j, bb : bb + 1],
                            Bt[:, j, bb : bb + 1],
                            op0=ALU.mult,
                            op1=ALU.add,
                        )

                for bb in range(PAIR):
                    b = ip * PAIR + bb
                    x_t = xts.pop(b)
                    y_t = yp.tile([P, NCH, HWD], BF16, tag="y")
                    yv = y_d[b].rearrange("(t p) f -> p t f", p=P)
                    p2("dve" if last else "act", y_t, x_t, 0, bb)
                    nc.scalar.dma_start(out=yv[:, 0:1, :], in_=y_t[:, 0:1, :])
                    p2("act" if last else "gp", y_t, x_t, 1, bb)
                    p2("gp", y_t, x_t, 2, bb)
                    p2("dve" if last else "gp", y_t, x_t, 3, bb)
                    nc.gpsimd.dma_start(out=yv[:, 1:4, :], in_=y_t[:, 1:4, :])

            # software pipeline: apply(ip-1) issues between the two samples
            # of stats(ip), so ACT/GpSimd pass2 (and the stores they gate)
            # stay fed while DVE grinds through the next pair's bn_stats,
            # and PE has a sample's worth of slack to finish logits.
            for ip in range(npair):
                stats_sample(2 * ip)
                if ip >= 1:
                    apply_phase(ip - 1)
                stats_sample(2 * ip + 1)
                stats_tail(ip)
            apply_phase(npair - 1)
    nc.finalize()
    return nc


def pack_consts(
    experts_weight, experts_bias, shared_weight, shared_bias, router_w, router_b
):
    ca = np.zeros((P, CA_W), np.float32)
    ca[:, 0:32] = (
        (np.ascontiguousarray(router_w.T) / 4.0)
        .reshape(NCH, P, E)
        .transpose(1, 0, 2)
        .reshape(P, 32)
    )
    pidx = np.arange(P)
    ca[:, 32:40] = (pidx[:, None] // CPG == np.arange(8)[None, :]).astype(np.float32)
    sw = shared_weight.reshape(NCH, P).T
    sb = shared_bias.reshape(NCH, P).T
    ca[:, 40:48] = np.repeat(sw, PAIR, axis=1)
    ca[:, 48:56] = np.repeat(sb, PAIR, axis=1)
    cb = np.zeros((64, CB_W), np.float32)
    for u, tab in enumerate([experts_weight, experts_bias]):
        for j in range(NCH):
            k = 4 * u + j
            cb[8 * k : 8 * k + 8, 0:P] = tab[:, j * P : (j + 1) * P]
    cb[0:8, P : 2 * P] = (np.arange(E)[:, None] == pidx[None, :] // CPG).astype(
        np.float32
    )
    for k in range(8):
        cb[8 * k : 8 * k + 8, 256 + 2 * k : 258 + 2 * k] = 1.0
    cb[0:1, 272:280] = router_b[None, :]
    cb[0:PAIR, 280:282] = np.eye(PAIR, dtype=np.float32)
    cb[0:1, 282:284] = 1.0
    return ca, cb


_NC_CACHE: dict[int, bass.Bass] = {}


def _get_nc(n_b: int) -> bass.Bass:
    if n_b not in _NC_CACHE:
        _NC_CACHE[n_b] = build(n_b)
    return _NC_CACHE[n_b]


def run(
    x,
    experts_weight,
    experts_bias,
    shared_weight,
    shared_bias,
    router_w,
    router_b,
    trace: bool = False,
    tmpdir=None,
):
    x = np.ascontiguousarray(np.asarray(x, np.float32)).reshape(B, C, HWD)
    ca, cb = pack_consts(
        np.asarray(experts_weight, np.float32),
        np.asarray(experts_bias, np.float32),
        np.asarray(shared_weight, np.float32),
        np.asarray(shared_bias, np.float32),
        np.asarray(router_w, np.float32),
        np.asarray(router_b, np.float32),
    )
    nc = _get_nc(BPC)
    in_maps = [
        {"x": x[i * BPC : (i + 1) * BPC], "ca": ca, "cb": cb} for i in range(NCORES)
    ]
    res = run_bass_kernel_spmd(
        nc, in_maps, list(range(NCORES)), trace=trace, tmpdir=tmpdir
    )
    y = np.concatenate(
        [np.asarray(res.results[i]["y"]).astype(np.float32) for i in range(NCORES)],
        axis=0,
    )
    return y.reshape(B, C, 32, 32), res


def kernel(**inputs) -> np.ndarray:
    y, _ = run(**inputs)
    return y
